# revision 43
# baseline (speedup 1.0000x reference)
"""Trainium2 Bass kernel for an AttentionBlock (GroupNorm + single-head
self-attention + projection + residual) over inputs x[8, 64, 64, 256].

Sharding: data-parallel over batch — one sample per NeuronCore (8 cores).
Each core runs an identical SPMD program on its own x[b] slice; the small
CxC weights are replicated.

Per-core dataflow (N=4096 tokens, C=256 channels), fp8 DoubleRow edition:
  1. GroupNorm(1 group) stats on DVE (rsqrt via Newton iterations — no ACT
     Sqrt, so the exp table set stays resident the whole kernel); the
     per-channel scale A=rstd*gamma folds into fp8 copies of wq/wk/wv rows;
     the shift B routes through the weights into the q bias (the k-bias
     shifts scores per-query only and cancels in softmax; the v-bias passes
     through the attention and folds into bp_eff = bva@wp + bp).
  2. Transpose x to channel-major hT8 [128c, 2, 4096tok] on the PE (bf16
     transpose-mode matmuls fed by DVE bf16 casts), fp8 on the PSUM->SBUF
     copy (ACT).
  3. Projections as fp8 DoubleRow matmuls (K=256 contraction in one
     instruction at 0.5 cycles/row). K first (one 2-bank PSUM tile + one
     DVE copy per slab), then V (one 2-bank tile + one ACT copy per slab):
     attention starts right after the K drain; the V drain overlaps the
     first chunk's scores, and the Q projections ride along inside the
     attention loop one chunk ahead.
  4. Attention, one continuous software pipeline over (chunk, double key
     block) steps, 512-query chunks, keys-on-partitions:
       sT[128k, 1024] <- two DoubleRow matmuls (one per key block)
       e2T = exp(sT * C^-1/2)    one 1024-wide ACT op, fp8 out, spanning a
                                 2-bank PSUM tile (ACT is the bottleneck
                                 engine; everything else is kept off ACT)
       d[1, q]   += ones8.T  @ e2T   (DoubleRow)
       oU[c, q]  += v8.T     @ e2T   (DoubleRow)
       out = (oU_bf16 @ wp_bf) * (1/d)[tok] + bp_eff + x
     1/d is taken token-major ([128, 4] via a DRAM round-trip transpose)
     so the reciprocal is 4 columns instead of 512 on one partition.
     Softmax max-subtraction is skipped: |scaled scores| < 5 for this
     operator's scale, so exp <= 150 fits fp8e4 (max 240) and fp32.
"""

import numpy as np

import concourse.bass as bass
import concourse.tile as tile
from concourse import bacc
from concourse import mybir
from concourse.bass_utils import run_bass_kernel_spmd
from concourse.masks import make_identity

F32 = mybir.dt.float32
F32R = mybir.dt.float32r
F8 = mybir.dt.float8e4
BF16 = mybir.dt.bfloat16
I32 = mybir.dt.int32
AF = mybir.ActivationFunctionType
OP = mybir.AluOpType
DR = mybir.MatmulPerfMode.DoubleRow

N = 4096          # tokens per sample (64*64)
C = 256           # channels
P = 128           # partitions
KC = C // P       # 2 channel chunks
TB = N // P       # 32 token blocks
QCW = 512         # query-chunk width
NQC = N // QCW    # 8 query chunks
NDJ = TB // 2     # 16 double key blocks
NSLAB = N // 512  # 8 slabs of 512 tokens
EPS = 1e-3
SCALE = float(C) ** -0.5
B = 8


def _r(ap):
    return ap.bitcast(F32R)


def _bpart(ap, parts=P):
    """Broadcast a 1-D (or [1, w]) AP across `parts` partitions."""
    inner = list(ap.ap)
    if len(inner) > 1 and inner[0][1] == 1:
        inner = inner[1:]
    return bass.AP(tensor=ap.tensor, offset=ap.offset, ap=[[0, parts]] + inner)


def build(nc: bass.Bass):
    x = nc.dram_tensor("x", [N, C], F32, kind="ExternalInput")
    w_dram = {
        name: nc.dram_tensor(name, [C, C], F32, kind="ExternalInput")
        for name in ("wq", "wk", "wv", "wp")
    }
    b_dram = {
        name: nc.dram_tensor(name, [C], F32, kind="ExternalInput")
        for name in ("bq", "bk", "bv", "bp", "gamma", "beta")
    }
    out = nc.dram_tensor("out", [N, C], F32, kind="ExternalOutput")
    d_dram = nc.dram_tensor("d_scratch", [NQC, QCW], F32, kind="Internal")
    bva_dram = nc.dram_tensor("bva_scratch", [C], F32, kind="Internal")

    with tile.TileContext(nc) as tc:
        with (
            tc.tile_pool(name="const", bufs=1) as const,
            tc.tile_pool(name="small", bufs=2) as small,
            tc.tile_pool(name="big", bufs=1) as big,
        ):
            # ---- input DMAs ----------------------------------------------
            # On-chip token order is the PERMUTED enumeration tok = 32p + j
            # (partition-major): attention is permutation-invariant over
            # tokens, and this makes each partition's x slice one contiguous
            # 32KB DRAM read (128 big descriptors instead of 4096 1KB ones).
            # Only the load and the final store ever see DRAM token order.
            x_nat = big.tile([P, TB, C], F32, tag="x_nat")
            x_re = x[:, :].rearrange("(p j) c -> p j c", j=TB)
            out_re = out[:, :].rearrange("(p j) c -> p j c", j=TB)
            qs = (nc.sync, nc.gpsimd, nc.scalar)
            for g in range(8):
                qs[g % 3].dma_start(
                    out=x_nat[:, 4 * g:4 * (g + 1), :],
                    in_=x_re[:, 4 * g:4 * (g + 1), :],
                )
            w_sb = {}
            for name in ("wq", "wk", "wv", "wp"):
                t = const.tile([P, KC, C], F32, tag=f"w_{name}")
                nc.sync.dma_start(
                    out=t,
                    in_=w_dram[name][:, :].rearrange("(kc p) n -> p kc n", p=P),
                )
                w_sb[name] = t
            bias_p = {}
            for name in ("bq", "gamma", "beta"):
                t = const.tile([P, KC], F32, tag=f"p_{name}")
                nc.gpsimd.dma_start(
                    out=t, in_=b_dram[name][:].rearrange("(kc p) -> p kc", p=P)
                )
                bias_p[name] = t
            bp1 = const.tile([1, C], F32, tag="bp1")
            nc.gpsimd.dma_start(out=bp1, in_=_bpart(b_dram["bp"][:], parts=1))
            bv1 = const.tile([1, C], F32, tag="bv1")
            nc.gpsimd.dma_start(out=bv1, in_=_bpart(b_dram["bv"][:], parts=1))

            # ---- replicated constants -------------------------------------
            ident = const.tile([P, P], F32, tag="ident")
            make_identity(nc, ident)
            ident_bf = const.tile([P, P], BF16, tag="ident_bf")
            nc.vector.tensor_copy(out=ident_bf, in_=ident)
            ones_mat = const.tile([P, P], F32, tag="ones_mat")
            nc.vector.memset(ones_mat, 1.0 / P)
            ones1 = const.tile([1, P], F32, tag="ones1")
            nc.vector.memset(ones1, 1.0)
            # dual-fp8 LDWEIGHTS needs the pair-dim step 16B-aligned, so
            # the ones column is padded out to stride 16.
            ones8 = const.tile([P, 2, 16], F8, tag="ones8")
            nc.vector.memset(ones8, 1.0)

            qT = big.tile([P, KC, N], F8, tag="qT")
            kT = big.tile([P, KC, N], F8, tag="kT")
            v8 = big.tile([P, TB, C], F8, tag="v8")
            hT8 = big.tile([P, KC, N], F8, tag="hT8")
            x_bf = big.tile([P, TB, C], BF16, tag="x_bf")
            w8 = {
                name: const.tile([P, KC, C], F8, tag=f"w8_{name}",
                                 name=f"w8_{name}")
                for name in ("wq", "wk", "wv")
            }
            wp_bf = const.tile([P, KC, C], BF16, tag="wp_bf")

            # ---- phases 1-3: stats, transpose, K/V projections ------------
            with (
                tc.tile_pool(name="psm", bufs=1, space="PSUM") as psm,
                tc.tile_pool(name="pst", bufs=3, space="PSUM") as pst,
                tc.tile_pool(name="ps23", bufs=4, space="PSUM") as ps23,
            ):
                # dummy transpose reading only `ident`: absorbs the Pool-sem
                # wait on the PE so real transposes carry a single DMA wait
                # (transpose-mode LDWEIGHTS supports only one sync wait).
                dummy_ps = psm.tile([P, P], F32, tag="misc")
                nc.tensor.matmul(
                    dummy_ps, lhsT=ident, rhs=ident, is_transpose=True,
                    start=True, stop=True,
                )
                # first ACT instruction: load the exp table set (contains
                # Copy too) once, overlapped with the x DMA; Sqrt is avoided
                # entirely so no other set is ever loaded
                dummy_e = small.tile([1, 1], F32, tag="dummy_e")
                nc.scalar.activation(out=dummy_e, in_=ones1[0:1, 0:1],
                                     func=AF.Exp)

                # fp8 weights scaled by gamma only: the scalar rstd of the
                # 1-group GroupNorm moves into the exp scale (rstd^2) and
                # into wp_bf (rstd), so K/V/Q projections have NO stats
                # dependency and start as soon as transposes land.
                for name in ("wk", "wv", "wq"):
                    for kc in range(KC):
                        nc.vector.tensor_scalar_mul(
                            out=w8[name][:, kc, :],
                            in0=w_sb[name][:, kc, :],
                            scalar1=bias_p["gamma"][:, kc:kc + 1],
                        )

                # per slab: bn_stats + bf16 cast (both DVE, paced only by the
                # x DMA), then bf16 transposes (PE) drained to fp8 hT8 (ACT)
                x512 = x_nat[:].rearrange("p a b -> p (a b)").rearrange(
                    "p (s f) -> p s f", f=512
                )
                stats = small.tile([P, 16, 6], F32, tag="stats")

                def x_cast(g):
                    # x -> bf16 on ACT (idle here); one slab ahead of its
                    # transposes so the in-order ACT queue never stalls
                    nc.scalar.activation(
                        out=x_bf[:, 4 * g:4 * (g + 1), :].rearrange(
                            "p a b -> p (a b)"
                        ),
                        in_=x_nat[:, 4 * g:4 * (g + 1), :].rearrange(
                            "p a b -> p (a b)"
                        ),
                        func=AF.Copy,
                    )

                late_kt = []
                x_cast(0)
                for g in range(NSLAB):
                    if g + 1 < NSLAB:
                        x_cast(g + 1)
                    for h in range(2):
                        nc.vector.bn_stats(
                            out=stats[:, 2 * g + h, :],
                            in_=x512[:, 2 * g + h, :],
                        )
                    for kc in range(KC):
                        pt = pst.tile([P, 512], BF16, tag="trans")
                        for t in range(4):
                            tb = g * 4 + t
                            nc.tensor.matmul(
                                pt[:, t * P:(t + 1) * P],
                                lhsT=x_bf[:, tb, kc * P:(kc + 1) * P],
                                rhs=ident_bf,
                                is_transpose=True,
                                start=(t == 0),
                                stop=(t == 3),
                                skip_group_check=True,
                            )
                        nc.scalar.activation(
                            out=hT8[:, kc, g * 512:(g + 1) * 512],
                            in_=pt,
                            func=AF.Copy,
                        )
                    # K projection for this slab (k-bias shifts scores
                    # per-query only -> cancels in softmax: pure cast copy);
                    # one 1-bank tile per output chunk -> 2 slabs of
                    # lookahead before a copy blocks the next projection
                    for co in range(KC):
                        pq = ps23.tile([P, 512], F32, tag="proj",
                                       name=f"pq{co}")
                        nc.tensor.matmul(
                            pq,
                            lhsT=w8["wk"][:, :, co * P:(co + 1) * P],
                            rhs=hT8[:, :, g * 512:(g + 1) * 512],
                            perf_mode=DR,
                            start=True,
                            stop=True,
                            skip_group_check=True,
                        )
                        if g < NSLAB - 2:
                            nc.vector.tensor_copy(
                                out=kT[:, co, g * 512:(g + 1) * 512], in_=pq
                            )
                        else:
                            late_kt.append(((g, co), pq))

                # GroupNorm stats -> A (all DVE except one PE column-sum)
                mv = small.tile([P, 2], F32, tag="mv")
                nc.vector.bn_aggr(out=mv, in_=stats)
                # msq = [mean_p, var_p + mean_p^2]
                msq = small.tile([P, 2], F32, tag="msq")
                nc.vector.tensor_copy(out=msq[:, 0:1], in_=mv[:, 0:1])
                nc.vector.tensor_tensor(
                    out=msq[:, 1:2], in0=mv[:, 0:1], in1=mv[:, 0:1], op=OP.mult
                )
                nc.vector.tensor_tensor(
                    out=msq[:, 1:2], in0=msq[:, 1:2], in1=mv[:, 1:2], op=OP.add
                )
                # ones_mat(1/P) matmul: replicated column means
                pstat = psm.tile([P, 2], F32, tag="misc")
                nc.tensor.matmul(pstat, lhsT=ones_mat, rhs=msq, start=True,
                                 stop=True)
                # st = [mean, E[x^2], var, var+eps] on every partition
                st = small.tile([P, 4], F32, tag="st")
                nc.vector.tensor_copy(out=st[:, 0:2], in_=pstat)
                nc.vector.tensor_tensor(
                    out=st[:, 2:3], in0=st[:, 0:1], in1=st[:, 0:1], op=OP.mult
                )
                nc.vector.tensor_tensor(
                    out=st[:, 2:3], in0=st[:, 1:2], in1=st[:, 2:3],
                    op=OP.subtract,
                )
                nc.vector.tensor_scalar_add(
                    out=st[:, 3:4], in0=st[:, 2:3], scalar1=EPS
                )
                # rstd = rsqrt(var+eps) via magic-constant + 2 Newton steps,
                # entirely on DVE: avoids the ACT Sqrt (and its 1.3us table
                # load + the exp-table reload it would force later)
                magic = small.tile([P, 1], F32, tag="magic")
                nc.vector.memset(magic, 1.3211836172961054e19)  # 0x5f3759df
                rstd = small.tile([P, 1], F32, tag="rstd")
                nt = small.tile([P, 1], F32, tag="nt")
                nc.vector.tensor_scalar(
                    out=rstd.bitcast(I32), in0=st[:, 3:4].bitcast(I32),
                    scalar1=1, scalar2=None, op0=OP.arith_shift_right,
                )
                nc.vector.tensor_tensor(
                    out=rstd.bitcast(I32), in0=magic.bitcast(I32),
                    in1=rstd.bitcast(I32), op=OP.subtract,
                )
                for _ in range(2):
                    nc.vector.tensor_tensor(out=nt, in0=rstd, in1=rstd,
                                            op=OP.mult)
                    nc.vector.tensor_tensor(out=nt, in0=nt, in1=st[:, 3:4],
                                            op=OP.mult)
                    nc.vector.tensor_scalar(out=nt, in0=nt, scalar1=-0.5,
                                            scalar2=1.5, op0=OP.mult,
                                            op1=OP.add)
                    nc.vector.tensor_tensor(out=rstd, in0=rstd, in1=nt,
                                            op=OP.mult)
                # A = rstd*gamma, Bc = beta - mean*A  (h = A*x + Bc)
                Ab = small.tile([P, KC], F32, tag="Ab")
                Bb = small.tile([P, KC], F32, tag="Bb")
                nc.vector.tensor_scalar_mul(out=Ab, in0=bias_p["gamma"],
                                            scalar1=rstd)
                nc.vector.tensor_scalar_mul(out=Bb, in0=Ab,
                                            scalar1=st[:, 0:1])
                nc.vector.tensor_tensor(
                    out=Bb, in0=bias_p["beta"], in1=Bb, op=OP.subtract
                )
                # exp scale = C^-1/2 * rstd^2 (per-partition AP); wp picks
                # up the remaining rstd factor; the q bias is pre-divided by
                # rstd so the scores matmul stays exact.
                es = small.tile([P, 1], F32, tag="es")
                nc.vector.tensor_scalar(out=es, in0=rstd,
                                        scalar1=rstd[:, 0:1], scalar2=SCALE,
                                        op0=OP.mult, op1=OP.mult)
                inv_rstd = small.tile([P, 1], F32, tag="inv_rstd")
                nc.vector.reciprocal(out=inv_rstd, in_=rstd)
                for kc in range(KC):
                    nc.vector.tensor_scalar_mul(
                        out=wp_bf[:, kc, :], in0=w_sb["wp"][:, kc, :],
                        scalar1=rstd,
                    )

                # q-chunk-0 bias (B routed through wq) + projection
                pb = psm.tile([P, KC], F32, tag="misc", name="pb")
                for co in range(KC):
                    for kc in range(KC):
                        nc.tensor.matmul(
                            pb[:, co:co + 1],
                            lhsT=w_sb["wq"][:, kc, co * P:(co + 1) * P],
                            rhs=Bb[:, kc:kc + 1],
                            start=(co == 0 and kc == 0),
                            stop=(co == KC - 1 and kc == KC - 1),
                            skip_group_check=True,
                        )
                bq_adj = small.tile([P, KC], F32, tag="badj")
                nc.vector.tensor_tensor(
                    out=bq_adj, in0=pb, in1=bias_p["bq"], op=OP.add
                )
                nc.vector.tensor_scalar_mul(out=bq_adj, in0=bq_adj,
                                            scalar1=inv_rstd)
                for co in range(KC):
                    pq0 = psm.tile([P, 512], F32, tag="misc", name="pq0")
                    nc.tensor.matmul(
                        pq0,
                        lhsT=w8["wq"][:, :, co * P:(co + 1) * P],
                        rhs=hT8[:, :, 0:512],
                        perf_mode=DR,
                        start=True,
                        stop=True,
                        skip_group_check=True,
                    )
                    nc.vector.tensor_scalar_add(
                        out=qT[:, co, 0:512],
                        in0=pq0,
                        scalar1=bq_adj[:, co:co + 1],
                    )
                for (g, co), pq_l in late_kt:
                    nc.vector.tensor_copy(
                        out=kT[:, co, g * 512:(g + 1) * 512], in_=pq_l
                    )

                # v-bias passes through attention (weights sum to 1), so it
                # folds into the output bias: bp_eff = bva @ wp + bp.
                # bva needs the channel-partitioned layout -> DRAM round-trip.
                pbv = psm.tile([1, C], F32, tag="misc")
                for kc in range(KC):
                    nc.tensor.matmul(
                        pbv,
                        lhsT=Bb[:, kc:kc + 1],
                        rhs=w_sb["wv"][:, kc, :],
                        start=(kc == 0),
                        stop=(kc == KC - 1),
                    )
                bva1 = small.tile([1, C], F32, tag="bva1")
                nc.vector.tensor_tensor(
                    out=bva1, in0=pbv[0:1, :], in1=bv1[0:1, :], op=OP.add
                )
                nc.sync.dma_start(out=bva_dram[:], in_=bva1[0:1, :])
                bva_pkc = small.tile([P, KC], F32, tag="bva_pkc")
                nc.sync.dma_start(
                    out=bva_pkc,
                    in_=bva_dram[:].rearrange("(kc p) -> p kc", p=P),
                )
                pbp = psm.tile([1, C], F32, tag="misc")
                for kc in range(KC):
                    nc.tensor.matmul(
                        pbp,
                        lhsT=bva_pkc[:, kc:kc + 1],
                        rhs=w_sb["wp"][:, kc, :],
                        start=(kc == 0),
                        stop=(kc == KC - 1),
                    )
                bpe1 = small.tile([1, C], F32, tag="bpe1")
                nc.vector.tensor_tensor(
                    out=bpe1, in0=pbp[0:1, :], in1=bp1[0:1, :], op=OP.add
                )
                pbpe = psm.tile([P, C], F32, tag="misc")
                nc.tensor.matmul(pbpe, lhsT=ones1, rhs=bpe1, start=True,
                                 stop=True)
                bp_eff = small.tile([P, C], F32, tag="bp_eff")
                nc.vector.tensor_copy(out=bp_eff, in_=pbpe)

            # ---- phase 4: attention, one continuous software pipeline ----
            # Flattened over (chunk, double-key-block) steps: the scores+exp
            # stream leads the PV/denominator stream by LAG steps and flows
            # across chunk boundaries, so neither the PE nor ACT drains at a
            # chunk edge.  Q projections ride along one chunk ahead, sharing
            # the out-projection PSUM bank.
            with (
                tc.tile_pool(name="epool", bufs=6) as epool,
                tc.tile_pool(name="opool", bufs=3) as opool,
                tc.tile_pool(name="rpool", bufs=3) as rpool,
                tc.tile_pool(name="ps_s", bufs=2, space="PSUM") as ps_s,
                tc.tile_pool(name="ps_pv", bufs=2, space="PSUM") as ps_pv,
                tc.tile_pool(name="ps_d", bufs=1, space="PSUM") as ps_d,
                tc.tile_pool(name="ps_p", bufs=1, space="PSUM") as ps_p,
            ):
                def v_proj(g):
                    for half in range(2):
                        tb0 = 4 * g + 2 * half
                        pv = ps_p.tile([P, 512], F32, tag="pp", name="pvj")
                        for u in range(2):
                            nc.tensor.matmul(
                                pv[:, u * C:(u + 1) * C],
                                lhsT=hT8[:, :, (tb0 + u) * P:(tb0 + u + 1) * P],
                                rhs=w8["wv"][:],
                                perf_mode=DR,
                                start=True,
                                stop=True,
                                skip_group_check=True,
                            )
                        nc.vector.tensor_copy(
                            out=v8[:, tb0:tb0 + 2, :].rearrange(
                                "p a b -> p (a b)"
                            ),
                            in_=pv,
                        )

                def q_proj(g):
                    for co in range(KC):
                        pq = ps_p.tile([P, 512], F32, tag="pp", name="pq")
                        nc.tensor.matmul(
                            pq,
                            lhsT=w8["wq"][:, :, co * P:(co + 1) * P],
                            rhs=hT8[:, :, g * 512:(g + 1) * 512],
                            perf_mode=DR,
                            start=True,
                            stop=True,
                        )
                        nc.vector.tensor_scalar_add(
                            out=qT[:, co, g * 512:(g + 1) * 512],
                            in0=pq,
                            scalar1=bq_adj[:, co:co + 1],
                        )

                def tail_chunk(qc, rdT, oU, pool=None):
                    """out-projection on unnormalized bf16 oU, then normalize
                    with the token-major 1/d scalars in the residual chain
                    (emitted one chunk later so the PE never waits on the
                    normalize chain)."""
                    for t in range(QCW // P):
                        tb = qc * (QCW // P) + t
                        pool_, tag_ = (pool, "pv") if pool else (ps_p, "pp")
                        pp = pool_.tile([P, C], F32, tag=tag_, name="pp")
                        for kc in range(KC):
                            nc.tensor.matmul(
                                pp,
                                lhsT=oU[:, kc, t * P:(t + 1) * P],
                                rhs=wp_bf[:, kc, :],
                                start=(kc == 0),
                                stop=(kc == KC - 1),
                            )
                        res = rpool.tile([P, C], F32, tag="res")
                        nc.vector.tensor_scalar_mul(
                            out=res, in0=pp, scalar1=rdT[:, t:t + 1]
                        )
                        nc.vector.tensor_tensor(
                            out=res, in0=res, in1=bp_eff, op=OP.add
                        )
                        nc.vector.tensor_tensor(
                            out=res, in0=res, in1=x_nat[:, tb, :], op=OP.add
                        )
                        eng = nc.sync if t % 2 == 0 else nc.gpsimd
                        eng.dma_start(out=out_re[:, tb, :], in_=res)

                LAG = 2
                NSTEP = NQC * NDJ
                elist = {}
                po = pd = None
                pending = None
                for step in range(NSTEP + LAG):
                    if step < NSTEP:
                        qc_s, dj_s = divmod(step, NDJ)
                        qsl = slice(qc_s * QCW, (qc_s + 1) * QCW)
                        ps = ps_s.tile([P, 2 * QCW], F32, tag="sT")
                        for half in range(2):
                            j = 2 * dj_s + half
                            nc.tensor.matmul(
                                ps[:, half * QCW:(half + 1) * QCW],
                                lhsT=kT[:, :, j * P:(j + 1) * P],
                                rhs=qT[:, :, qsl],
                                perf_mode=DR,
                                start=True,
                                stop=True,
                                skip_group_check=True,
                            )
                        # next-chunk Q / next-slab V projections AFTER this
                        # step's scores so they fill PE slack instead of
                        # delaying the exp
                        if 1 <= step <= NSLAB:
                            v_proj(step - 1)
                        if dj_s == 8 and qc_s + 1 < NQC:
                            q_proj(qc_s + 1)
                        e2 = epool.tile([P, 2, QCW], F8, tag="eT")
                        nc.scalar.activation(
                            out=e2[:].rearrange("p a b -> p (a b)"),
                            in_=ps,
                            func=AF.Exp,
                            scale=es[:, 0:1],
                        )
                        elist[step] = e2
                    if step >= LAG:
                        pv_step = step - LAG
                        qc_v, dj_v = divmod(pv_step, NDJ)
                        if dj_v == 0:
                            po = [
                                ps_pv.tile([P, QCW], F32, tag="pv",
                                           name=f"pv{_co}")
                                for _co in range(KC)
                            ]
                            pd = ps_d.tile([1, QCW], F32, tag="pd")
                        e2 = elist.pop(pv_step)
                        for co in range(KC):
                            nc.tensor.matmul(
                                po[co],
                                lhsT=v8[:, 2 * dj_v:2 * dj_v + 2,
                                        co * P:(co + 1) * P],
                                rhs=e2[:],
                                perf_mode=DR,
                                start=(dj_v == 0),
                                stop=(dj_v == NDJ - 1),
                            )
                        nc.tensor.matmul(
                            pd,
                            lhsT=ones8[:, :, 0:1],
                            rhs=e2[:],
                            perf_mode=DR,
                            start=(dj_v == 0),
                            stop=(dj_v == NDJ - 1),
                        )
                        if dj_v == NDJ - 1:
                            # drain PV/d PSUM: bf16 copy; d to token-major
                            # [128, 4] via a DRAM round-trip so the
                            # reciprocal is 4 columns, not 512
                            d_sb = rpool.tile([1, QCW], F32, tag="d_sb")
                            nc.vector.tensor_copy(out=d_sb, in_=pd)
                            nc.sync.dma_start(out=d_dram[qc_v, :],
                                              in_=d_sb[0:1, :])
                            oU = opool.tile([P, KC, QCW], BF16, tag="oU")
                            for co in range(KC):
                                nc.vector.tensor_copy(out=oU[:, co, :],
                                                      in_=po[co])
                            dT = rpool.tile([P, QCW // P], F32, tag="dT")
                            nc.gpsimd.dma_start(
                                out=dT,
                                in_=d_dram[qc_v, :].rearrange(
                                    "(t p) -> p t", p=P
                                ),
                            )
                            rdT = rpool.tile([P, QCW // P], F32, tag="rdT")
                            nc.vector.reciprocal(out=rdT, in_=dT)
                            if pending is not None:
                                tail_chunk(*pending)
                            pending = (qc_v, rdT, oU)
                tail_chunk(*pending, pool=ps_pv)

    return nc


_CACHE = {}


def _get_nc():
    if "nc" not in _CACHE:
        nc = bacc.Bacc()
        build(nc)
        nc.compile()
        _CACHE["nc"] = nc
    return _CACHE["nc"]


def _in_maps(inputs):
    x = np.asarray(inputs["x"], dtype=np.float32)
    shared = {
        k: np.ascontiguousarray(np.asarray(inputs[k], dtype=np.float32))
        for k in ("wq", "bq", "wk", "bk", "wv", "bv", "wp", "bp", "gamma", "beta")
    }
    maps = []
    for b in range(B):
        m = dict(shared)
        m["x"] = np.ascontiguousarray(x[b].reshape(N, C))
        maps.append(m)
    return maps


def run(inputs, trace=False):
    nc = _get_nc()
    res = run_bass_kernel_spmd(
        nc, _in_maps(inputs), core_ids=list(range(B)), trace=trace
    )
    outs = np.stack(
        [res.results[b]["out"].reshape(64, 64, C) for b in range(B)], axis=0
    )
    return outs, res


def kernel(**inputs) -> np.ndarray:
    outs, _ = run(inputs, trace=False)
    return outs
